# revision 38
# baseline (speedup 1.0000x reference)
import functools
import sys

import numpy as np

sys.path.insert(0, "/opt/trn_rl_repo")

import concourse.bass as bass
import concourse.bacc as bacc
import concourse.mybir as mybir
from concourse import tile

B, L, D = 256, 256, 64
NCORES = 8
BP = B // NCORES
LM = L - 1
N2 = 2 * LM
NBLK = 17
BLK = LM // NBLK
XF = BP * 2 * D
CROWS = BP * L
TROWS = CROWS + BP * L // 32
F32 = mybir.dt.float32
F16 = mybir.dt.float16
I16 = mybir.dt.int16
U8 = mybir.dt.uint8
ADD = mybir.AluOpType.add
MULT = mybir.AluOpType.mult
EQ = mybir.AluOpType.is_equal
AND = mybir.AluOpType.bitwise_and
SHR = mybir.AluOpType.logical_shift_right
COPY = mybir.ActivationFunctionType.Copy


def _build_program():
    nc = bacc.Bacc(None, target_bir_lowering=False)
    xin_d = nc.declare_dram_parameter("xin", [TROWS, D], U8, isOutput=False)
    yin_d = nc.declare_dram_parameter("yin", [TROWS, D], U8, isOutput=False)
    out_d = nc.declare_dram_parameter("out", [BP, 1], F32, isOutput=True)
    A_d = nc.dram_tensor("A_scratch", [BP, LM, LM], F32)

    with tile.TileContext(nc) as tc:
        with (
            tc.tile_pool(name="const", bufs=1) as cpool,
            tc.tile_pool(name="ps", bufs=2, space="PSUM") as pspool,
            tc.tile_pool(name="ev", bufs=3) as evpool,
            tc.tile_pool(name="pde", bufs=1) as upool,
            tc.tile_pool(name="ablk", bufs=2) as apool,
            tc.tile_pool(name="tmp", bufs=2) as tpool,
        ):
            def load_unpack(in_d, tag):
                lo_d = in_d[0:CROWS, :]
                loq = cpool.tile([128, XF], U8, name=f"loq_{tag}")
                nc.gpsimd.dma_start(
                    out=loq[:].rearrange("q (p c d) -> q p c d", p=BP, c=2),
                    in_=lo_d.rearrange("(p c q) d -> q p c d", p=BP, c=2),
                )
                sclt = cpool.tile([128, 2 * BP * 2], U8, name=f"sclt_{tag}")
                nc.gpsimd.dma_start(
                    out=sclt[:].rearrange("q (p c t) -> q p c t", p=BP, c=2),
                    in_=in_d[CROWS:TROWS, :]
                    .rearrange("a b -> (a b)")
                    .rearrange("(p c q t) -> q p c t", p=BP, c=2, t=2),
                )
                xf = cpool.tile([128, XF], F16, name=f"xf_{tag}")
                nc.scalar.activation(xf[:], loq[:], COPY, bias=-128.0)
                scl16 = sclt[:].bitcast(F16)
                sbc = bass.AP(
                    scl16.tensor,
                    scl16.offset,
                    [scl16.ap[0], [scl16.ap[1][0], BP * 2], [0, D]],
                )
                xq = cpool.tile([128, XF], F16, name=f"xq_{tag}")
                nc.gpsimd.tensor_mul(xq[:], xf[:], sbc)
                return xq

            xq = load_unpack(xin_d, "x")
            yq = load_unpack(yin_d, "y")

            def x_ap(p, c):
                o = (p * 2 + c) * D
                return xq[:, o : o + D]

            def y_ap(p, c):
                o = (p * 2 + c) * D
                return yq[:, o : o + D]

            iot = cpool.tile([128, 2 * LM], I16)
            m1 = cpool.tile([128, 2 * LM], F16)
            m0 = cpool.tile([128, 2 * LM], F16)
            dTq = cpool.tile([128, 2 * LM], F16)
            nc.gpsimd.iota(iot[:], [[128, 2], [-1, LM]], base=0, channel_multiplier=1)
            nc.vector.tensor_scalar(m1[:], iot[:], 1, None, EQ)
            nc.vector.tensor_scalar(m0[:], iot[:], 0, None, EQ)
            nc.vector.tensor_sub(dTq[:], m1[:], m0[:])

            def dT_ap(c):
                return dTq[:, c * LM : (c + 1) * LM]

            for p in range(BP):
                dxT_ps = pspool.tile([D, LM], F32, tag="dxps", name="dxT_ps")
                dyT_ps = pspool.tile([D, LM], F32, tag="dyps", name="dyT_ps")
                for c in range(2):
                    nc.tensor.matmul(
                        dxT_ps[:], x_ap(p, c), dT_ap(c),
                        start=(c == 0), stop=(c == 1),
                    )
                for c in range(2):
                    nc.tensor.matmul(
                        dyT_ps[:], y_ap(p, c), dT_ap(c),
                        start=(c == 0), stop=(c == 1),
                    )
                dxT_sb = evpool.tile([D, LM], F32, tag="dxe", name="dxT_sb")
                dyT_sb = evpool.tile([D, LM], F32, tag="dye", name="dyT_sb")
                nc.scalar.activation(dxT_sb[:], dxT_ps[:], COPY, scale=0.5)
                nc.scalar.activation(dyT_sb[:], dyT_ps[:], COPY, scale=0.5)
                for m0_, m1_ in ((0, 128), (128, LM)):
                    a_ps = pspool.tile([128, LM], F32, tag="aps", name="a_ps")
                    nc.tensor.matmul(
                        a_ps[: m1_ - m0_, :], dxT_sb[:, m0_:m1_], dyT_sb[:],
                        start=True, stop=True,
                    )
                    a_sb = evpool.tile([128, LM], F32, tag="aev", name="a_sb", bufs=64)
                    nc.scalar.activation(
                        a_sb[: m1_ - m0_, :], a_ps[: m1_ - m0_, :], COPY, bias=-1.0
                    )
                    nc.sync.dma_start(out=A_d[p][m0_:m1_, :], in_=a_sb[: m1_ - m0_, :])

            u_bufs = [
                upool.tile([BP, N2 + 1], F32, tag=f"u{i}", name=f"u{i}")
                for i in range(2)
            ]
            nc.vector.memset(u_bufs[0][:], 1.0)
            nc.vector.memset(u_bufs[1][:], 1.0)
            step = 0
            for b in range(NBLK):
                ablk = apool.tile([BP, BLK * LM], F32, tag="ablk", name="ablk")
                nc.sync.dma_start(
                    out=ablk[:],
                    in_=A_d[:, b * BLK : (b + 1) * BLK, :].rearrange(
                        "p r a -> p (r a)"
                    ),
                )
                for r in range(BLK):
                    base = ablk[:, r * LM : (r + 1) * LM]
                    dbl = bass.AP(
                        base.tensor,
                        base.offset,
                        [base.ap[0], [base.ap[1][0], LM], [0, 2]],
                    )
                    for _ in range(2):
                        up = u_bufs[step % 2]
                        un = u_bufs[(step + 1) % 2]
                        tmp = tpool.tile([BP, N2], F32, tag="tmp", name="tmp")
                        nc.gpsimd.tensor_mul(tmp[:], up[:, 0:N2], dbl)
                        nc.vector.tensor_tensor_scan(
                            un[:, 1 : N2 + 1], up[:, 1 : N2 + 1], tmp[:],
                            1.0, ADD, ADD,
                        )
                        step += 1
            nc.sync.dma_start(out=out_d[:], in_=u_bufs[step % 2][:, N2 : N2 + 1])
    nc.compile()
    return nc


@functools.lru_cache(maxsize=1)
def _program():
    return _build_program()


@functools.lru_cache(maxsize=1)
def _executor():
    import jax
    from jax.sharding import Mesh, PartitionSpec
    from jax.experimental.shard_map import shard_map
    from concourse import bass2jax
    from concourse.bass2jax import _bass_exec_p, install_neuronx_cc_hook

    nc = _program()
    install_neuronx_cc_hook()
    partition_name = (
        nc.partition_id_tensor.name if nc.partition_id_tensor is not None else None
    )
    in_names: list[str] = []
    out_names: list[str] = []
    out_avals = []
    zero_specs = []
    for alloc in nc.m.functions[0].allocations:
        if not isinstance(alloc, mybir.MemoryLocationSet):
            continue
        name = alloc.memorylocations[0].name
        if alloc.kind == "ExternalInput":
            if name != partition_name:
                in_names.append(name)
        elif alloc.kind == "ExternalOutput":
            shape = tuple(alloc.tensor_shape)
            dtype = mybir.dt.np(alloc.dtype)
            out_names.append(name)
            out_avals.append(jax.core.ShapedArray(shape, dtype))
            zero_specs.append((shape, dtype))
    n_params = len(in_names)
    n_outs = len(out_avals)
    in_names_all = in_names + out_names + (
        [partition_name] if partition_name else []
    )
    donate = tuple(range(n_params, n_params + n_outs))

    def _body(*args):
        operands = list(args)
        if partition_name is not None:
            operands.append(bass2jax.partition_id_tensor())
        outs = _bass_exec_p.bind(
            *operands,
            out_avals=tuple(out_avals),
            in_names=tuple(in_names_all),
            out_names=tuple(out_names),
            lowering_input_output_aliases=(),
            sim_require_finite=True,
            sim_require_nnan=True,
            nc=nc,
        )
        return tuple(outs)

    devices = jax.devices()[:NCORES]
    assert len(devices) == NCORES
    mesh = Mesh(np.asarray(devices), ("core",))
    in_specs = (PartitionSpec("core"),) * (n_params + n_outs)
    out_specs = (PartitionSpec("core"),) * len(out_names)
    sharded = jax.jit(
        shard_map(
            _body, mesh=mesh, in_specs=in_specs, out_specs=out_specs,
            check_rep=False,
        ),
        donate_argnums=donate,
        keep_unused=True,
    )
    return sharded, in_names, out_names, zero_specs


_C_SRC = r"""
#include <stdint.h>
#include <math.h>
#include <immintrin.h>

/* x: [ncores][32][256*64] f32 -> out: per core 32*256*64 lo bytes (natural
   order) then 256 rows x 64 of per-(pair,l) f16 scales in (p c q t) order.
   Per row of 64: scale = f16(rowmax/127); m = round(x/scale) + 128 in
   [0,255]. */
void quant8(const float* x, uint8_t* out, long ncores, long core_stride) {
    const long PL = 256 * 64;
    for (long c = 0; c < ncores; c++) {
        const float* xc = x + c * 32 * PL;
        uint8_t* loc = out + c * core_stride;
        uint8_t* sc = loc + 32 * PL;
        for (long p = 0; p < 32; p++) {
            for (long l = 0; l < 256; l++) {
                const float* r = xc + p * PL + l * 64;
                uint8_t* lo = loc + (p * 256 + l) * 64;
                float mx = 0.f;
                for (int d = 0; d < 64; d++) {
                    float v = fabsf(r[d]);
                    mx = v > mx ? v : mx;
                }
                unsigned short hb = _cvtss_sh(mx * (1.0f / 127.0f),
                                              _MM_FROUND_TO_NEAREST_INT);
                float s16 = _cvtsh_ss(hb);
                if (s16 == 0.f) {
                    hb = _cvtss_sh(1.0f, _MM_FROUND_TO_NEAREST_INT);
                    s16 = 1.0f;
                }
                float k = 1.0f / s16;
                for (int d = 0; d < 64; d++) {
                    int m = (int)(r[d] * k + 128.5f);
                    m = m < 0 ? 0 : (m > 255 ? 255 : m);
                    lo[d] = (uint8_t)m;
                }
                long so = ((p * 2 + (l >> 7)) * 128 + (l & 127)) * 2;
                sc[so] = (uint8_t)(hb & 255);
                sc[so + 1] = (uint8_t)(hb >> 8);
            }
        }
    }
}
"""


def _build_cquant():
    import ctypes
    import hashlib
    import os
    import subprocess

    h = hashlib.md5(_C_SRC.encode()).hexdigest()[:12]
    so = f"/tmp/_sigq_{h}.so"
    if not os.path.exists(so):
        cpath = f"/tmp/_sigq_{h}.c"
        with open(cpath, "w") as f:
            f.write(_C_SRC)
        tmp = so + f".{os.getpid()}.tmp"
        subprocess.run(
            ["gcc", "-O3", "-march=native", "-funroll-loops", "-shared",
             "-fPIC", cpath, "-o", tmp, "-lm"],
            check=True, capture_output=True,
        )
        os.replace(tmp, so)
    lib = ctypes.CDLL(so)
    lib.quant8.restype = None
    lib.quant8.argtypes = [
        ctypes.c_void_p, ctypes.c_void_p, ctypes.c_long, ctypes.c_long,
    ]
    return lib


try:
    _clib = _build_cquant()
except Exception:
    _clib = None


def _scale_rows(v: float) -> np.ndarray:
    return np.full((128,), v, np.float32).view(np.uint8).reshape(8, D)


def _quant8_c(arr: np.ndarray):
    a = np.ascontiguousarray(arr, np.float32).reshape(-1)
    comb = np.empty(NCORES * TROWS * D, np.uint8)
    _clib.quant8(a.ctypes.data, comb.ctypes.data, NCORES, TROWS * D)
    return comb.reshape(NCORES * TROWS, D)


def _pack_fallback(arr: np.ndarray):
    a = np.ascontiguousarray(arr, np.float32).reshape(NCORES, BP, L, D)
    rowmax = np.abs(a).max(axis=3, keepdims=True)
    s32 = (rowmax / 127.0).astype(np.float16).astype(np.float32)
    s32[s32 == 0.0] = 1.0
    s16 = s32.astype(np.float16)
    m = np.clip(np.rint(a / s32).astype(np.int32) + 128, 0, 255)
    comb = np.empty((NCORES, TROWS, D), np.uint8)
    comb[:, 0:CROWS] = m.astype(np.uint8).reshape(NCORES, BP * L, D)
    sv = s16.reshape(NCORES, BP, 2, 128).view(np.uint8)
    comb[:, CROWS:TROWS] = (
        sv.reshape(NCORES, BP, 2, 128, 2).reshape(NCORES, TROWS - CROWS, D)
    )
    return comb.reshape(NCORES * TROWS, D)


@functools.lru_cache(maxsize=1)
def _sharding():
    import jax
    from jax.sharding import Mesh, NamedSharding, PartitionSpec

    mesh = Mesh(np.asarray(jax.devices()[:NCORES]), ("core",))
    return NamedSharding(mesh, PartitionSpec("core"))


def kernel(xs: np.ndarray, ys: np.ndarray) -> np.ndarray:
    import jax

    sharded, in_names, out_names, zero_specs = _executor()
    sh = _sharding()
    quant = _quant8_c if _clib is not None else _pack_fallback
    feeds = {"xin": jax.device_put(quant(np.asarray(xs)), sh)}
    feeds["yin"] = jax.device_put(quant(np.asarray(ys)), sh)
    concat_in = [feeds[name] for name in in_names]
    concat_zeros = [
        np.zeros((NCORES * s[0], *s[1:]), dt) for s, dt in zero_specs
    ]
    out_arrs = sharded(*concat_in, *concat_zeros)
    out = np.asarray(out_arrs[out_names.index("out")])
    return out.reshape(B).astype(np.float32, copy=False)


# revision 39
# speedup vs baseline: 1.0336x; 1.0336x over previous
import functools
import sys

import numpy as np

sys.path.insert(0, "/opt/trn_rl_repo")

import concourse.bass as bass
import concourse.bacc as bacc
import concourse.mybir as mybir
from concourse import tile

B, L, D = 256, 256, 64
NCORES = 8
BP = B // NCORES
LM = L - 1
N2 = 2 * LM
NBLK = 17
BLK = LM // NBLK
XF = BP * 2 * D
CROWS = BP * L
TROWS = CROWS + BP * L // 32
F32 = mybir.dt.float32
F16 = mybir.dt.float16
I16 = mybir.dt.int16
U8 = mybir.dt.uint8
ADD = mybir.AluOpType.add
MULT = mybir.AluOpType.mult
EQ = mybir.AluOpType.is_equal
AND = mybir.AluOpType.bitwise_and
SHR = mybir.AluOpType.logical_shift_right
COPY = mybir.ActivationFunctionType.Copy


def _build_program():
    nc = bacc.Bacc(None, target_bir_lowering=False)
    xin_d = nc.declare_dram_parameter("xin", [TROWS, D], U8, isOutput=False)
    yin_d = nc.declare_dram_parameter("yin", [TROWS, D], U8, isOutput=False)
    out_d = nc.declare_dram_parameter("out", [BP, 1], F32, isOutput=True)
    A_d = nc.dram_tensor("A_scratch", [BP, LM, LM], F32)

    with tile.TileContext(nc) as tc:
        with (
            tc.tile_pool(name="const", bufs=1) as cpool,
            tc.tile_pool(name="ps", bufs=2, space="PSUM") as pspool,
            tc.tile_pool(name="ev", bufs=3) as evpool,
            tc.tile_pool(name="pde", bufs=1) as upool,
            tc.tile_pool(name="ablk", bufs=2) as apool,
            tc.tile_pool(name="tmp", bufs=2) as tpool,
        ):
            def load_unpack(in_d, tag):
                lo_d = in_d[0:CROWS, :]
                loq = cpool.tile([128, XF], U8, name=f"loq_{tag}")
                nc.gpsimd.dma_start(
                    out=loq[:].rearrange("q (p c d) -> q p c d", p=BP, c=2),
                    in_=lo_d.rearrange("(p c q) d -> q p c d", p=BP, c=2),
                )
                sclt = cpool.tile([128, 2 * BP * 2], U8, name=f"sclt_{tag}")
                nc.gpsimd.dma_start(
                    out=sclt[:].rearrange("q (p c t) -> q p c t", p=BP, c=2),
                    in_=in_d[CROWS:TROWS, :]
                    .rearrange("a b -> (a b)")
                    .rearrange("(p c q t) -> q p c t", p=BP, c=2, t=2),
                )
                xf = cpool.tile([128, XF], F16, name=f"xf_{tag}")
                nc.scalar.activation(xf[:], loq[:], COPY, bias=-128.0)
                scl16 = sclt[:].bitcast(F16)
                sbc = bass.AP(
                    scl16.tensor,
                    scl16.offset,
                    [scl16.ap[0], [scl16.ap[1][0], BP * 2], [0, D]],
                )
                xq = cpool.tile([128, XF], F16, name=f"xq_{tag}")
                nc.gpsimd.tensor_mul(xq[:], xf[:], sbc)
                return xq

            xq = load_unpack(xin_d, "x")
            yq = load_unpack(yin_d, "y")

            def x_ap(p, c):
                o = (p * 2 + c) * D
                return xq[:, o : o + D]

            def y_ap(p, c):
                o = (p * 2 + c) * D
                return yq[:, o : o + D]

            iot = cpool.tile([128, 2 * LM], I16)
            m1 = cpool.tile([128, 2 * LM], F16)
            m0 = cpool.tile([128, 2 * LM], F16)
            dTq = cpool.tile([128, 2 * LM], F16)
            nc.gpsimd.iota(iot[:], [[128, 2], [-1, LM]], base=0, channel_multiplier=1)
            nc.vector.tensor_scalar(m1[:], iot[:], 1, None, EQ)
            nc.vector.tensor_scalar(m0[:], iot[:], 0, None, EQ)
            nc.vector.tensor_sub(dTq[:], m1[:], m0[:])

            def dT_ap(c):
                return dTq[:, c * LM : (c + 1) * LM]

            for p in range(BP):
                dxT_ps = pspool.tile([D, LM], F32, tag="dxps", name="dxT_ps")
                dyT_ps = pspool.tile([D, LM], F32, tag="dyps", name="dyT_ps")
                for c in range(2):
                    nc.tensor.matmul(
                        dxT_ps[:], x_ap(p, c), dT_ap(c),
                        start=(c == 0), stop=(c == 1),
                    )
                for c in range(2):
                    nc.tensor.matmul(
                        dyT_ps[:], y_ap(p, c), dT_ap(c),
                        start=(c == 0), stop=(c == 1),
                    )
                dxT_sb = evpool.tile([D, LM], F32, tag="dxe", name="dxT_sb")
                dyT_sb = evpool.tile([D, LM], F32, tag="dye", name="dyT_sb")
                nc.scalar.activation(dxT_sb[:], dxT_ps[:], COPY, scale=0.5)
                nc.scalar.activation(dyT_sb[:], dyT_ps[:], COPY, scale=0.5)
                for m0_, m1_ in ((0, 128), (128, LM)):
                    a_ps = pspool.tile([128, LM], F32, tag="aps", name="a_ps")
                    nc.tensor.matmul(
                        a_ps[: m1_ - m0_, :], dxT_sb[:, m0_:m1_], dyT_sb[:],
                        start=True, stop=True,
                    )
                    a_sb = evpool.tile([128, LM], F32, tag="aev", name="a_sb", bufs=64)
                    nc.scalar.activation(
                        a_sb[: m1_ - m0_, :], a_ps[: m1_ - m0_, :], COPY, bias=-1.0
                    )
                    nc.sync.dma_start(out=A_d[p][m0_:m1_, :], in_=a_sb[: m1_ - m0_, :])

            u_bufs = [
                upool.tile([BP, N2 + 1], F32, tag=f"u{i}", name=f"u{i}")
                for i in range(2)
            ]
            nc.vector.memset(u_bufs[0][:], 1.0)
            nc.vector.memset(u_bufs[1][:], 1.0)
            step = 0
            for b in range(NBLK):
                ablk = apool.tile([BP, BLK * LM], F32, tag="ablk", name="ablk")
                nc.sync.dma_start(
                    out=ablk[:],
                    in_=A_d[:, b * BLK : (b + 1) * BLK, :].rearrange(
                        "p r a -> p (r a)"
                    ),
                )
                for r in range(BLK):
                    base = ablk[:, r * LM : (r + 1) * LM]
                    dbl = bass.AP(
                        base.tensor,
                        base.offset,
                        [base.ap[0], [base.ap[1][0], LM], [0, 2]],
                    )
                    for _ in range(2):
                        up = u_bufs[step % 2]
                        un = u_bufs[(step + 1) % 2]
                        tmp = tpool.tile([BP, N2], F32, tag="tmp", name="tmp")
                        nc.gpsimd.tensor_mul(tmp[:], up[:, 0:N2], dbl)
                        nc.vector.tensor_tensor_scan(
                            un[:, 1 : N2 + 1], up[:, 1 : N2 + 1], tmp[:],
                            1.0, ADD, ADD,
                        )
                        step += 1
            nc.sync.dma_start(out=out_d[:], in_=u_bufs[step % 2][:, N2 : N2 + 1])
    nc.compile()
    return nc


@functools.lru_cache(maxsize=1)
def _program():
    return _build_program()


@functools.lru_cache(maxsize=1)
def _executor():
    import jax
    from jax.sharding import Mesh, PartitionSpec
    from jax.experimental.shard_map import shard_map
    from concourse import bass2jax
    from concourse.bass2jax import _bass_exec_p, install_neuronx_cc_hook

    nc = _program()
    install_neuronx_cc_hook()
    partition_name = (
        nc.partition_id_tensor.name if nc.partition_id_tensor is not None else None
    )
    in_names: list[str] = []
    out_names: list[str] = []
    out_avals = []
    zero_specs = []
    for alloc in nc.m.functions[0].allocations:
        if not isinstance(alloc, mybir.MemoryLocationSet):
            continue
        name = alloc.memorylocations[0].name
        if alloc.kind == "ExternalInput":
            if name != partition_name:
                in_names.append(name)
        elif alloc.kind == "ExternalOutput":
            shape = tuple(alloc.tensor_shape)
            dtype = mybir.dt.np(alloc.dtype)
            out_names.append(name)
            out_avals.append(jax.core.ShapedArray(shape, dtype))
            zero_specs.append((shape, dtype))
    n_params = len(in_names)
    n_outs = len(out_avals)
    in_names_all = in_names + out_names + (
        [partition_name] if partition_name else []
    )
    donate = tuple(range(n_params, n_params + n_outs))

    def _body(*args):
        operands = list(args)
        if partition_name is not None:
            operands.append(bass2jax.partition_id_tensor())
        outs = _bass_exec_p.bind(
            *operands,
            out_avals=tuple(out_avals),
            in_names=tuple(in_names_all),
            out_names=tuple(out_names),
            lowering_input_output_aliases=(),
            sim_require_finite=True,
            sim_require_nnan=True,
            nc=nc,
        )
        return tuple(outs)

    devices = jax.devices()[:NCORES]
    assert len(devices) == NCORES
    mesh = Mesh(np.asarray(devices), ("core",))
    in_specs = (PartitionSpec("core"),) * (n_params + n_outs)
    out_specs = (PartitionSpec("core"),) * len(out_names)
    sharded = jax.jit(
        shard_map(
            _body, mesh=mesh, in_specs=in_specs, out_specs=out_specs,
            check_rep=False,
        ),
        donate_argnums=donate,
        keep_unused=True,
    )
    return sharded, in_names, out_names, zero_specs


_C_SRC = r"""
#include <stdint.h>
#include <math.h>
#include <immintrin.h>

/* x: [ncores][32][256*64] f32 -> out: per core 32*256*64 lo bytes (natural
   order) then 256 rows x 64 of per-(pair,l) f16 scales in (p c q t) order.
   Per row of 64: scale = f16(rowmax/127); m = round(x/scale) + 128 in
   [0,255]. */
void quant8(const float* x, uint8_t* out, long ncores, long core_stride) {
    const long PL = 256 * 64;
    for (long c = 0; c < ncores; c++) {
        const float* xc = x + c * 32 * PL;
        uint8_t* loc = out + c * core_stride;
        uint8_t* sc = loc + 32 * PL;
        for (long p = 0; p < 32; p++) {
            for (long l = 0; l < 256; l++) {
                const float* r = xc + p * PL + l * 64;
                uint8_t* lo = loc + (p * 256 + l) * 64;
                float mx = 0.f;
                for (int d = 0; d < 64; d++) {
                    float v = fabsf(r[d]);
                    mx = v > mx ? v : mx;
                }
                unsigned short hb = _cvtss_sh(mx * (1.0f / 127.0f),
                                              _MM_FROUND_TO_NEAREST_INT);
                float s16 = _cvtsh_ss(hb);
                if (s16 == 0.f) {
                    hb = _cvtss_sh(1.0f, _MM_FROUND_TO_NEAREST_INT);
                    s16 = 1.0f;
                }
                float k = 1.0f / s16;
                for (int d = 0; d < 64; d++) {
                    int m = (int)(r[d] * k + 128.5f);
                    m = m < 0 ? 0 : (m > 255 ? 255 : m);
                    lo[d] = (uint8_t)m;
                }
                long so = ((p * 2 + (l >> 7)) * 128 + (l & 127)) * 2;
                sc[so] = (uint8_t)(hb & 255);
                sc[so + 1] = (uint8_t)(hb >> 8);
            }
        }
    }
}
"""


def _build_cquant():
    import ctypes
    import hashlib
    import os
    import subprocess

    h = hashlib.md5(_C_SRC.encode()).hexdigest()[:12]
    so = f"/tmp/_sigq_{h}.so"
    if not os.path.exists(so):
        cpath = f"/tmp/_sigq_{h}.c"
        with open(cpath, "w") as f:
            f.write(_C_SRC)
        tmp = so + f".{os.getpid()}.tmp"
        subprocess.run(
            ["gcc", "-O3", "-march=native", "-funroll-loops", "-shared",
             "-fPIC", cpath, "-o", tmp, "-lm"],
            check=True, capture_output=True,
        )
        os.replace(tmp, so)
    lib = ctypes.CDLL(so)
    lib.quant8.restype = None
    lib.quant8.argtypes = [
        ctypes.c_void_p, ctypes.c_void_p, ctypes.c_long, ctypes.c_long,
    ]
    return lib


try:
    _clib = _build_cquant()
except Exception:
    _clib = None


def _scale_rows(v: float) -> np.ndarray:
    return np.full((128,), v, np.float32).view(np.uint8).reshape(8, D)


def _quant8_c(arr: np.ndarray):
    a = np.ascontiguousarray(arr, np.float32).reshape(-1)
    comb = np.empty(NCORES * TROWS * D, np.uint8)
    _clib.quant8(a.ctypes.data, comb.ctypes.data, NCORES, TROWS * D)
    return comb.reshape(NCORES * TROWS, D)


def _pack_fallback(arr: np.ndarray):
    a = np.ascontiguousarray(arr, np.float32).reshape(NCORES, BP, L, D)
    rowmax = np.abs(a).max(axis=3, keepdims=True)
    s32 = (rowmax / 127.0).astype(np.float16).astype(np.float32)
    s32[s32 == 0.0] = 1.0
    s16 = s32.astype(np.float16)
    m = np.clip(np.rint(a / s32).astype(np.int32) + 128, 0, 255)
    comb = np.empty((NCORES, TROWS, D), np.uint8)
    comb[:, 0:CROWS] = m.astype(np.uint8).reshape(NCORES, BP * L, D)
    sv = s16.reshape(NCORES, BP, 2, 128).view(np.uint8)
    comb[:, CROWS:TROWS] = (
        sv.reshape(NCORES, BP, 2, 128, 2).reshape(NCORES, TROWS - CROWS, D)
    )
    return comb.reshape(NCORES * TROWS, D)


@functools.lru_cache(maxsize=1)
def _sharding():
    import jax
    from jax.sharding import Mesh, NamedSharding, PartitionSpec

    mesh = Mesh(np.asarray(jax.devices()[:NCORES]), ("core",))
    return NamedSharding(mesh, PartitionSpec("core"))


def kernel(xs: np.ndarray, ys: np.ndarray) -> np.ndarray:
    import jax

    sharded, in_names, out_names, zero_specs = _executor()
    sh = _sharding()
    quant = _quant8_c if _clib is not None else _pack_fallback
    feeds = {"xin": jax.device_put(quant(np.asarray(xs)), sh)}
    feeds["yin"] = jax.device_put(quant(np.asarray(ys)), sh)
    concat_in = [feeds[name] for name in in_names]
    concat_zeros = [
        np.zeros((NCORES * s[0], *s[1:]), dt) for s, dt in zero_specs
    ]
    out_arrs = sharded(*concat_in, *concat_zeros)
    out = np.asarray(out_arrs[out_names.index("out")])
    return out.reshape(B).astype(np.float32, copy=False)
